# revision 3
# baseline (speedup 1.0000x reference)
"""Trainium2 Bass kernel for nn_Actor (LSTM + LayerNorm + MLP + Gaussian head).

Sharding: data-parallel over batch. 8 cores x 32 batch each, no collectives.
All compute feature-major (transposed): partition axis = feature dims.

Per core (BL=32):
  Phase gx  (windowed, overlapped): gx.T = W_ihb @ [x;1].T per 8-step window,
            kept in SBUF (no DRAM round trip).
  Phase LSTM: 256 sequential steps. gates.T in PSUM [128, 4 blocks, 128]:
            block q packs [i|f|o|g] x 32 batch for h-dims 128q..128q+127.
            Stationary lhsT = W_hh.T tiles, moving rhs = masked h.T.
  Phase win (per window, hides under LSTM PE time): LayerNorm (stats via
            ones-matmul partition reduce), MLP with ELU (exp/relu compose,
            the -1 folded into the next layer's bias), Gaussian heads,
            logp/ent via a [16,2] reduction matmul.
"""

import numpy as np

import concourse.bass as bass
import concourse.bacc as bacc
import concourse.mybir as mybir
import concourse.tile as tile
from concourse.bass_utils import run_bass_kernel_spmd

AF = mybir.ActivationFunctionType
OP = mybir.AluOpType
F32 = mybir.dt.float32
BF16 = mybir.dt.bfloat16

T, B, OBS, H, A = 256, 256, 32, 512, 8
NCORES = 8
BL = B // NCORES            # 32
M1, M2 = 512, 256
LOG2PI = float(np.log(2.0 * np.pi))
EPS = 1e-5
W = 8                        # steps per window
WC = W * BL                  # 256 cols per window

# gate row blocks in original (i,f,g,o) order -> our per-block order [i,f,o,g]
GATE_BASE = (0, 512, 1536, 1024)

DT_W = F32                   # weight/activation matmul dtype (flip to BF16 for v2)


def _gate_perm():
    p = np.empty(4 * H, dtype=np.int64)
    n = 0
    for q in range(H // 128):
        for base in GATE_BASE:
            p[n:n + 128] = base + 128 * q + np.arange(128)
            n += 128
    return p


def _np_dt(dt):
    return np.float32 if dt == F32 else np.dtype("bfloat16") if hasattr(np, "bfloat16") else np.float32


def build(nt: int, dt_w=DT_W):
    """Build the per-core Bass graph for nt timesteps."""
    assert nt % W == 0
    nw = nt // W
    ntbl = nt * BL

    nc = bacc.Bacc("TRN2", target_bir_lowering=False, debug=False,
                   num_devices=NCORES)

    def di(name, shape, dt=F32):
        return nc.dram_tensor(name, list(shape), dt, kind="ExternalInput").ap()

    def do(name, shape, dt=F32):
        return nc.dram_tensor(name, list(shape), dt, kind="ExternalOutput").ap()

    x_d = di("xaugT", [33, ntbl], dt_w)
    mk_d = di("maskb", [nt, 128, 128])
    a_d = di("aT", [A, ntbl])
    h0_d = di("h0T", [128, 4, BL])
    c0_d = di("c0T", [128, 4, BL])
    whh_d = di("whhT", [128, 4, 4 * H], dt_w)
    wih_d = di("wihbT", [33, 4 * H], dt_w)
    w1_d = di("w1T", [128, 4, M1], dt_w)
    w2_d = di("w2T", [128, 4, M2], dt_w)
    wms_d = di("wmsT", [128, 2, 64], dt_w)
    lng_d = di("lng", [128, 4])
    lnb_d = di("lnb", [128, 4])
    b1_d = di("b1", [128, 4])
    nb1_d = di("nb1", [128, 4])
    b2_d = di("b2", [128, 2])
    nb2_d = di("nb2", [128, 2])
    csta_d = di("cst_a", [64, 2])
    cstb_d = di("cst_b", [1, 64], dt_w)
    cstc_d = di("cst_c", [2, 1])

    le_o = do("le", [2, ntbl])
    mean_o = do("meanT", [A, ntbl])
    std_o = do("stdT", [A, ntbl])
    hT_o = do("hT", [128, 4, BL])
    cT_o = do("cT", [128, 4, BL])

    with tile.TileContext(nc) as tc:
        with (
            tc.tile_pool(name="const", bufs=1) as pc,
            tc.tile_pool(name="gxw", bufs=2) as pgxw,
            tc.tile_pool(name="hsw", bufs=2) as phsw,
            tc.tile_pool(name="state", bufs=2) as pst,
            tc.tile_pool(name="step", bufs=3) as pstep,
            tc.tile_pool(name="mask", bufs=6) as pmk,
            tc.tile_pool(name="big", bufs=1) as pbig,
            tc.tile_pool(name="small", bufs=2) as psm,
            tc.tile_pool(name="io", bufs=3) as pio,
            tc.tile_pool(name="psg", bufs=2, space="PSUM") as psg,
            tc.tile_pool(name="psgx", bufs=2, space="PSUM") as psgx,
            tc.tile_pool(name="psw", bufs=2, space="PSUM") as psw,
        ):
            # ---- persistent constants ----
            whh = pc.tile([128, 4, 4 * H], dt_w)
            nc.sync.dma_start(whh[:], whh_d)
            wih = pc.tile([33, 4 * H], dt_w)
            nc.sync.dma_start(wih[:], wih_d)
            w1s = pc.tile([128, 4, M1], dt_w)
            nc.sync.dma_start(w1s[:], w1_d)
            w2s = pc.tile([128, 4, M2], dt_w)
            nc.sync.dma_start(w2s[:], w2_d)
            wms = pc.tile([128, 2, 64], dt_w)
            nc.sync.dma_start(wms[:], wms_d)
            lng = pc.tile([128, 4], F32)
            nc.sync.dma_start(lng[:], lng_d)
            lnb = pc.tile([128, 4], F32)
            nc.sync.dma_start(lnb[:], lnb_d)
            b1s = pc.tile([128, 4], F32)
            nc.sync.dma_start(b1s[:], b1_d)
            nb1s = pc.tile([128, 4], F32)
            nc.sync.dma_start(nb1s[:], nb1_d)
            b2s = pc.tile([128, 2], F32)
            nc.sync.dma_start(b2s[:], b2_d)
            nb2s = pc.tile([128, 2], F32)
            nc.sync.dma_start(nb2s[:], nb2_d)
            csta = pc.tile([64, 2], F32)
            nc.sync.dma_start(csta[:], csta_d)
            cstb = pc.tile([1, 64], dt_w)
            nc.sync.dma_start(cstb[:], cstb_d)
            cstc = pc.tile([2, 1], F32)
            nc.sync.dma_start(cstc[:], cstc_d)
            h0 = pc.tile([128, 4, BL], F32)
            nc.sync.dma_start(h0[:], h0_d)
            c0 = pc.tile([128, 4, BL], F32)
            nc.sync.dma_start(c0[:], c0_d)

            ones_mu = pc.tile([128, 1], F32)
            nc.vector.memset(ones_mu[:], 1.0 / H)
            ones_bc = pc.tile([1, 128], F32)
            nc.vector.memset(ones_bc[:], 1.0)
            ones_row = pc.tile([1, WC], dt_w)
            nc.vector.memset(ones_row[:], 1.0)

            # ---- gx window fill: gx.T = W_ihb @ [x;1].T  -> SBUF ----
            def gx_fill(w):
                gxw = pgxw.tile([128, W, 512], dt_w, tag="gxw")
                xa = pio.tile([33, WC], dt_w, tag="xa")
                nc.sync.dma_start(xa[:], x_d[:, w * WC:(w + 1) * WC])
                for m in range(16):
                    ps = psgx.tile([128, WC], F32, tag="gx")
                    nc.tensor.matmul(ps[:], wih[:, bass.ts(m, 128)], xa[:],
                                     start=True, stop=True)
                    nc.scalar.copy(
                        gxw[:, :, bass.ts(m, 32)],
                        ps[:].rearrange("p (s b) -> p s b", s=W))
                return gxw

            # ---- one LSTM step ----
            def lstm_step(t, h_prev, c_prev, gxw, hsw):
                toff = t % W
                mk = pmk.tile([128, 128], F32, tag="mk")
                nc.sync.dma_start(mk[:], mk_d[t])
                mk3 = mk[:].rearrange("p (q b) -> p q b", q=4)
                ht = pstep.tile([128, 4, BL], dt_w, tag="ht")
                nc.vector.tensor_tensor(ht[:], h_prev, mk3, OP.mult)
                ct = pstep.tile([128, 4, BL], F32, tag="ct")
                nc.vector.tensor_tensor(ct[:], c_prev, mk3, OP.mult)

                G = psg.tile([128, 4, 128], F32, tag="G")
                for m in range(16):
                    q, r = divmod(m, 4)
                    for k in range(4):
                        nc.tensor.matmul(
                            G[:, q, bass.ts(r, 32)],
                            whh[:, k, bass.ts(m, 128)],
                            ht[:, k, :],
                            start=(k == 0), stop=(k == 3))

                act = pstep.tile([128, 4, 128], F32, tag="act")
                gx3 = gxw[:, toff, :].rearrange("p (q j) -> p q j", q=4)
                nc.vector.tensor_tensor(act[:], G[:], gx3, OP.add)
                nc.scalar.activation(act[:, :, 0:96], act[:, :, 0:96], AF.Sigmoid)
                nc.scalar.activation(act[:, :, 96:128], act[:, :, 96:128], AF.Tanh)
                i_s = act[:, :, 0:32]
                f_s = act[:, :, 32:64]
                o_s = act[:, :, 64:96]
                tg = act[:, :, 96:128]

                t1 = pstep.tile([128, 4, BL], F32, tag="t1")
                nc.vector.tensor_tensor(t1[:], f_s, ct[:], OP.mult)
                t2 = pstep.tile([128, 4, BL], F32, tag="t2")
                nc.vector.tensor_tensor(t2[:], i_s, tg, OP.mult)
                c_new = pst.tile([128, 4, BL], F32, tag="c")
                nc.vector.tensor_tensor(c_new[:], t1[:], t2[:], OP.add)
                tcx = pstep.tile([128, 4, BL], F32, tag="tc")
                nc.scalar.activation(tcx[:], c_new[:], AF.Tanh)
                h_slot = hsw[:, :, toff, :]
                nc.vector.tensor_tensor(h_slot, o_s, tcx[:], OP.mult)
                return h_slot, c_new

            # ---- per-window LayerNorm + MLP + heads ----
            def win_phase(w, hsw):
                # LN stats via ones-matmul partition reduce
                sq = pbig.tile([128, 4, W, BL], F32, tag="sq")
                for k in range(4):
                    nc.scalar.activation(sq[:, k], hsw[:, k], AF.Square)
                mu = psw.tile([1, WC], F32, tag="w")
                for k in range(4):
                    nc.tensor.matmul(mu[:], ones_mu[:],
                                     hsw[:, k].rearrange("p s b -> p (s b)"),
                                     start=(k == 0), stop=(k == 3))
                ms = psw.tile([1, WC], F32, tag="w")
                for k in range(4):
                    nc.tensor.matmul(ms[:], ones_mu[:],
                                     sq[:, k].rearrange("p s b -> p (s b)"),
                                     start=(k == 0), stop=(k == 3))
                murow = psm.tile([1, WC], F32, tag="murow")
                nc.scalar.copy(murow[:], mu[:])
                mu2 = psm.tile([1, WC], F32, tag="mu2")
                nc.scalar.square(mu2[:], mu[:])
                ve = psm.tile([1, WC], F32, tag="ve")
                nc.vector.tensor_tensor(ve[:], ms[:], mu2[:], OP.subtract)
                nc.vector.tensor_scalar(ve[:], ve[:], EPS, None, OP.add)
                rec = psm.tile([1, WC], F32, tag="rec")
                nc.vector.reciprocal(rec[:], ve[:])
                rstd = psm.tile([1, WC], F32, tag="rstd")
                nc.scalar.sqrt(rstd[:], rec[:])

                bcm = psw.tile([128, WC], F32, tag="w")
                nc.tensor.matmul(bcm[:], ones_bc[:], murow[:], start=True, stop=True)
                bcr = psw.tile([128, WC], F32, tag="w")
                nc.tensor.matmul(bcr[:], ones_bc[:], rstd[:], start=True, stop=True)

                hn = pbig.tile([128, 4, WC], dt_w, tag="hn")
                for k in range(4):
                    tmp = psm.tile([128, WC], F32, tag="lntmp")
                    nc.vector.tensor_tensor(
                        tmp[:], hsw[:, k].rearrange("p s b -> p (s b)"),
                        bcm[:], OP.subtract)
                    nc.vector.tensor_tensor(tmp[:], tmp[:], bcr[:], OP.mult)
                    nc.scalar.activation(hn[:, k], tmp[:], AF.Identity,
                                         bias=lnb[:, k:k + 1], scale=lng[:, k:k + 1])

                # MLP layer 1 (+ELU; output is elu(y)+1, -1 folded into b2')
                h1 = pbig.tile([128, 4, WC], dt_w, tag="h1")
                for m in range(4):
                    ps = psw.tile([128, WC], F32, tag="w")
                    for k in range(4):
                        nc.tensor.matmul(ps[:], w1s[:, k, bass.ts(m, 128)],
                                         hn[:, k], start=(k == 0), stop=(k == 3))
                    r = psm.tile([128, WC], F32, tag="er")
                    nc.scalar.activation(r[:], ps[:], AF.Relu, bias=b1s[:, m:m + 1])
                    r2 = psm.tile([128, WC], F32, tag="er2")
                    nc.scalar.activation(r2[:], ps[:], AF.Relu,
                                         bias=nb1s[:, m:m + 1], scale=-1.0)
                    e = psm.tile([128, WC], F32, tag="ee")
                    nc.scalar.activation(e[:], r2[:], AF.Exp, scale=-1.0)
                    nc.vector.tensor_tensor(h1[:, m], e[:], r[:], OP.add)

                # MLP layer 2
                h2 = pbig.tile([128, 2, WC], dt_w, tag="h2")
                for m in range(2):
                    ps = psw.tile([128, WC], F32, tag="w")
                    for k in range(4):
                        nc.tensor.matmul(ps[:], w2s[:, k, bass.ts(m, 128)],
                                         h1[:, k], start=(k == 0), stop=(k == 3))
                    r = psm.tile([128, WC], F32, tag="er")
                    nc.scalar.activation(r[:], ps[:], AF.Relu, bias=b2s[:, m:m + 1])
                    r2 = psm.tile([128, WC], F32, tag="er2")
                    nc.scalar.activation(r2[:], ps[:], AF.Relu,
                                         bias=nb2s[:, m:m + 1], scale=-1.0)
                    e = psm.tile([128, WC], F32, tag="ee")
                    nc.scalar.activation(e[:], r2[:], AF.Exp, scale=-1.0)
                    nc.vector.tensor_tensor(h2[:, m], e[:], r[:], OP.add)

                # heads: [mean; logstd_raw] = Wms @ h2' + b (bias via ones-row MM)
                hd = psw.tile([64, WC], F32, tag="w")
                for k in range(2):
                    nc.tensor.matmul(hd[:], wms[:, k], h2[:, k],
                                     start=(k == 0), stop=False)
                nc.tensor.matmul(hd[:], cstb[:], ones_row[:],
                                 start=False, stop=True)

                meanv = pio.tile([A, WC], F32, tag="mean")
                nc.scalar.copy(meanv[:], hd[0:A, :])
                stack = psm.tile([64, WC], F32, tag="stack")
                nc.vector.memset(stack[:], 0.0)
                nc.vector.tensor_scalar(stack[32:32 + A, :], hd[32:32 + A, :],
                                        -5.0, 2.0, OP.max, OP.min)
                stdv = pio.tile([A, WC], F32, tag="std")
                nc.scalar.activation(stdv[:], stack[32:32 + A, :], AF.Exp)
                rs = psm.tile([A, WC], F32, tag="rs")
                nc.scalar.activation(rs[:], stack[32:32 + A, :], AF.Exp, scale=-1.0)
                at = pio.tile([A, WC], F32, tag="at")
                nc.sync.dma_start(at[:], a_d[:, w * WC:(w + 1) * WC])
                z = psm.tile([A, WC], F32, tag="z")
                nc.vector.tensor_tensor(z[:], at[:], meanv[:], OP.subtract)
                nc.vector.tensor_tensor(z[:], z[:], rs[:], OP.mult)
                nc.scalar.activation(stack[0:A, :], z[:], AF.Square)

                le = psw.tile([2, WC], F32, tag="w")
                nc.tensor.matmul(le[:], csta[:], stack[:], start=True, stop=True)
                leo = pio.tile([2, WC], F32, tag="leo")
                nc.vector.tensor_scalar(leo[:], le[:], cstc[:, 0:1], None, OP.add)
                nc.sync.dma_start(le_o[:, w * WC:(w + 1) * WC], leo[:])
                nc.sync.dma_start(mean_o[:, w * WC:(w + 1) * WC], meanv[:])
                nc.sync.dma_start(std_o[:, w * WC:(w + 1) * WC], stdv[:])

            # ---- main emission ----
            gx_tiles = {0: gx_fill(0)}
            if nw > 1:
                gx_tiles[1] = gx_fill(1)
            h_prev, c_prev = h0[:], c0[:]
            for w in range(nw):
                hsw = phsw.tile([128, 4, W, BL], F32, tag="hsw")
                for toff in range(W):
                    t = w * W + toff
                    h_prev, c_new = lstm_step(t, h_prev, c_prev, gx_tiles[w], hsw)
                    c_prev = c_new[:]
                if w + 2 < nw:
                    gx_tiles[w + 2] = gx_fill(w + 2)
                    del gx_tiles[w]
                win_phase(w, hsw)
                if w == nw - 1:
                    nc.sync.dma_start(hT_o, h_prev)
                    nc.sync.dma_start(cT_o, c_prev)

    nc.compile()
    return nc


_BUILD_CACHE = {}


def _get_nc(nt, dt_w=DT_W):
    key = (nt, str(dt_w))
    if key not in _BUILD_CACHE:
        _BUILD_CACHE[key] = build(nt, dt_w)
    return _BUILD_CACHE[key]


def _prep_core_inputs(c, nt, x, done, h0, c0, action,
                      W_ih, W_hh, b_ih, b_hh, g_ln, b_ln,
                      W1, b1, W2, b2, Wm, bm, Ws, bs, dt_w=DT_W):
    perm = _gate_perm()
    npdt = np.float32
    bsl = slice(BL * c, BL * (c + 1))

    W_hh_re = W_hh[perm].astype(np.float32)
    W_ih_re = W_ih[perm].astype(np.float32)
    bias_re = (b_ih + b_hh)[perm].astype(np.float32)

    whhT = np.ascontiguousarray(
        W_hh_re.T.reshape(4, 128, 4 * H).transpose(1, 0, 2)).astype(npdt)
    wihbT = np.ascontiguousarray(
        np.concatenate([W_ih_re, bias_re[:, None]], 1).T).astype(npdt)

    xc = x[:nt, bsl, :]                                   # [nt, BL, OBS]
    xaugT = np.empty((33, nt * BL), dtype=npdt)
    xaugT[:32] = xc.transpose(2, 0, 1).reshape(OBS, nt * BL)
    xaugT[32] = 1.0

    mrow = (1.0 - done[:nt, bsl]).astype(np.float32)      # [nt, BL]
    maskb = np.broadcast_to(
        np.tile(mrow, (1, 4))[:, None, :], (nt, 128, 128))
    maskb = np.ascontiguousarray(maskb)

    aT = np.ascontiguousarray(
        action.reshape(T, B, A)[:nt, bsl].transpose(2, 0, 1).reshape(A, nt * BL))

    h0T = np.ascontiguousarray(
        h0[0, bsl].T.reshape(4, 128, BL).transpose(1, 0, 2))
    c0T = np.ascontiguousarray(
        c0[0, bsl].T.reshape(4, 128, BL).transpose(1, 0, 2))

    w1T = np.ascontiguousarray(
        W1.T.reshape(4, 128, M1).transpose(1, 0, 2)).astype(npdt)
    w2T = np.ascontiguousarray(
        W2.T.reshape(4, 128, M2).transpose(1, 0, 2)).astype(npdt)
    wms_pad = np.zeros((64, M2), dtype=np.float32)
    wms_pad[0:A] = Wm
    wms_pad[32:32 + A] = Ws
    wmsT = np.ascontiguousarray(
        wms_pad.T.reshape(2, 128, 64).transpose(1, 0, 2)).astype(npdt)

    lng = np.ascontiguousarray(g_ln.reshape(4, 128).T)
    lnb = np.ascontiguousarray(b_ln.reshape(4, 128).T)
    b1v = np.ascontiguousarray(b1.reshape(4, 128).T)
    b2p = b2 - W2.sum(1)                                   # fold elu +1 of h1
    b2v = np.ascontiguousarray(b2p.reshape(2, 128).T)
    bmp = bm - Wm.sum(1)                                   # fold elu +1 of h2
    bsp = bs - Ws.sum(1)

    cst_a = np.zeros((64, 2), dtype=np.float32)
    cst_a[0:A, 0] = -0.5
    cst_a[32:32 + A, 0] = -1.0
    cst_a[32:32 + A, 1] = 1.0
    cst_b = np.zeros((1, 64), dtype=np.float32)
    cst_b[0, 0:A] = bmp
    cst_b[0, 32:32 + A] = bsp
    cst_b = cst_b.astype(npdt)
    cst_c = np.array([[-0.5 * A * LOG2PI], [A * (0.5 + 0.5 * LOG2PI)]],
                     dtype=np.float32)

    return {
        "xaugT": xaugT, "maskb": maskb, "aT": aT,
        "h0T": h0T, "c0T": c0T,
        "whhT": whhT, "wihbT": wihbT,
        "w1T": w1T, "w2T": w2T, "wmsT": wmsT,
        "lng": lng, "lnb": lnb,
        "b1": b1v, "nb1": -b1v, "b2": b2v, "nb2": -b2v,
        "cst_a": cst_a, "cst_b": cst_b, "cst_c": cst_c,
    }


def run_device(nt, inputs, dt_w=DT_W, trace=False):
    """Shard, run on 8 cores, return per-core result dicts."""
    nc = _get_nc(nt, dt_w)
    in_maps = [
        _prep_core_inputs(c, nt, dt_w=dt_w, **inputs) for c in range(NCORES)
    ]
    res = run_bass_kernel_spmd(nc, in_maps, core_ids=list(range(NCORES)),
                               trace=trace)
    return res


def assemble(nt, inputs, results):
    """Gather per-core outputs into full reference-shaped outputs."""
    ntb = nt * B
    logp = np.empty((nt, B), dtype=np.float32)
    ent = np.empty((nt, B), dtype=np.float32)
    mean = np.empty((nt, B, A), dtype=np.float32)
    std = np.empty((nt, B, A), dtype=np.float32)
    h1 = np.empty((1, B, H), dtype=np.float32)
    c1 = np.empty((1, B, H), dtype=np.float32)
    for c in range(NCORES):
        bsl = slice(BL * c, BL * (c + 1))
        r = results[c]
        logp[:, bsl] = r["le"][0].reshape(nt, BL)
        ent[:, bsl] = r["le"][1].reshape(nt, BL)
        mean[:, bsl] = r["meanT"].reshape(A, nt, BL).transpose(1, 2, 0)
        std[:, bsl] = r["stdT"].reshape(A, nt, BL).transpose(1, 2, 0)
        h1[0, bsl] = r["hT"].transpose(2, 1, 0).reshape(BL, H)
        c1[0, bsl] = r["cT"].transpose(2, 1, 0).reshape(BL, H)
    action = np.asarray(inputs["action"], dtype=np.float32)[:ntb]
    return (action, logp.reshape(ntb), ent.reshape(ntb), (h1, c1),
            mean.reshape(ntb, A), std.reshape(ntb, A))


def kernel(**inputs):
    """Full unsharded inputs in, full reference-shaped output out."""
    inputs = {k: np.asarray(v) for k, v in inputs.items()}
    res = run_device(T, inputs)
    return assemble(T, inputs, res.results)


if __name__ == "__main__":
    rng = np.random.default_rng(0)
    ins = {
        "x": rng.standard_normal((T, B, OBS), dtype=np.float32),
        "done": (rng.random((T, B)) < 0.05).astype(np.float32),
        "h0": rng.standard_normal((1, B, H), dtype=np.float32) * 0.1,
        "c0": rng.standard_normal((1, B, H), dtype=np.float32) * 0.1,
        "action": rng.standard_normal((T * B, A), dtype=np.float32),
        "W_ih": rng.standard_normal((4 * H, OBS), dtype=np.float32) * 0.02,
        "W_hh": rng.standard_normal((4 * H, H), dtype=np.float32) * 0.02,
        "b_ih": np.zeros(4 * H, np.float32),
        "b_hh": np.zeros(4 * H, np.float32),
        "g_ln": np.ones(H, np.float32),
        "b_ln": np.zeros(H, np.float32),
        "W1": rng.standard_normal((M1, H), dtype=np.float32) * 0.02,
        "b1": np.zeros(M1, np.float32),
        "W2": rng.standard_normal((M2, M1), dtype=np.float32) * 0.02,
        "b2": np.zeros(M2, np.float32),
        "Wm": rng.standard_normal((A, M2), dtype=np.float32) * 0.01,
        "bm": np.zeros(A, np.float32),
        "Ws": rng.standard_normal((A, M2), dtype=np.float32) * 0.01,
        "bs": np.zeros(A, np.float32),
    }
    out = kernel(**ins)
    print("kernel ran; logp[:4] =", out[1][:4])


# revision 4
# speedup vs baseline: 1.1394x; 1.1394x over previous
"""Trainium2 Bass kernel for nn_Actor (LSTM + LayerNorm + MLP + Gaussian head).

Sharding: data-parallel over batch. 8 cores x 32 batch each, no collectives.
All compute feature-major (transposed): partition axis = feature dims.

Per core (BL=32):
  Phase gx  (windowed, overlapped): gx.T = W_ihb @ [x;1].T per 8-step window,
            kept in SBUF (no DRAM round trip).
  Phase LSTM: 256 sequential steps. gates.T in PSUM [128, 4 blocks, 128]:
            block q packs [i|f|o|g] x 32 batch for h-dims 128q..128q+127.
            Stationary lhsT = W_hh.T tiles, moving rhs = masked h.T.
  Phase win (per window, hides under LSTM PE time): LayerNorm (stats via
            ones-matmul partition reduce), MLP with ELU (exp/relu compose,
            the -1 folded into the next layer's bias), Gaussian heads,
            logp/ent via a [16,2] reduction matmul.
"""

import numpy as np

import concourse.bass as bass
import concourse.bacc as bacc
import concourse.mybir as mybir
import concourse.tile as tile
from concourse.bass_utils import run_bass_kernel_spmd

AF = mybir.ActivationFunctionType
OP = mybir.AluOpType
F32 = mybir.dt.float32
BF16 = mybir.dt.bfloat16

T, B, OBS, H, A = 256, 256, 32, 512, 8
NCORES = 8
BL = B // NCORES            # 32
M1, M2 = 512, 256
LOG2PI = float(np.log(2.0 * np.pi))
EPS = 1e-5
W = 8                        # steps per window
WC = W * BL                  # 256 cols per window

# gate row blocks in original (i,f,g,o) order -> our per-block order [i,f,o,g]
GATE_BASE = (0, 512, 1536, 1024)

DT_W = BF16                  # weight/activation matmul dtype


def _gate_perm():
    p = np.empty(4 * H, dtype=np.int64)
    n = 0
    for q in range(H // 128):
        for base in GATE_BASE:
            p[n:n + 128] = base + 128 * q + np.arange(128)
            n += 128
    return p


def _np_dt(dt):
    return np.float32 if dt == F32 else np.dtype("bfloat16") if hasattr(np, "bfloat16") else np.float32


def build(nt: int, dt_w=DT_W):
    """Build the per-core Bass graph for nt timesteps."""
    assert nt % W == 0
    nw = nt // W
    ntbl = nt * BL

    nc = bacc.Bacc("TRN2", target_bir_lowering=False, debug=False,
                   num_devices=NCORES)

    def di(name, shape, dt=F32):
        return nc.dram_tensor(name, list(shape), dt, kind="ExternalInput").ap()

    def do(name, shape, dt=F32):
        return nc.dram_tensor(name, list(shape), dt, kind="ExternalOutput").ap()

    x_d = di("xaugT", [33, ntbl], dt_w)
    mk_d = di("maskb", [nt, 128, 128])
    a_d = di("aT", [A, ntbl])
    h0_d = di("h0T", [128, 4, BL])
    c0_d = di("c0T", [128, 4, BL])
    whh_d = di("whhT", [128, 4, 4 * H], dt_w)
    wih_d = di("wihbT", [33, 4 * H], dt_w)
    w1_d = di("w1T", [128, 4, M1], dt_w)
    w2_d = di("w2T", [128, 4, M2], dt_w)
    wms_d = di("wmsT", [128, 2, 64], dt_w)
    lng_d = di("lng", [128, 4])
    lnb_d = di("lnb", [128, 4])
    b1_d = di("b1", [128, 4])
    nb1_d = di("nb1", [128, 4])
    b2_d = di("b2", [128, 2])
    nb2_d = di("nb2", [128, 2])
    csta_d = di("cst_a", [64, 2])
    cstb_d = di("cst_b", [1, 64], dt_w)
    cstc_d = di("cst_c", [2, 1])

    le_o = do("le", [2, ntbl])
    mean_o = do("meanT", [A, ntbl])
    std_o = do("stdT", [A, ntbl])
    hT_o = do("hT", [128, 4, BL])
    cT_o = do("cT", [128, 4, BL])

    with tile.TileContext(nc) as tc:
        with (
            tc.tile_pool(name="const", bufs=1) as pc,
            tc.tile_pool(name="gxw", bufs=2) as pgxw,
            tc.tile_pool(name="hsw", bufs=2) as phsw,
            tc.tile_pool(name="state", bufs=2) as pst,
            tc.tile_pool(name="step", bufs=3) as pstep,
            tc.tile_pool(name="mask", bufs=6) as pmk,
            tc.tile_pool(name="big", bufs=1) as pbig,
            tc.tile_pool(name="small", bufs=2) as psm,
            tc.tile_pool(name="io", bufs=3) as pio,
            tc.tile_pool(name="psg", bufs=2, space="PSUM") as psg,
            tc.tile_pool(name="psgx", bufs=2, space="PSUM") as psgx,
            tc.tile_pool(name="psw", bufs=2, space="PSUM") as psw,
        ):
            # ---- persistent constants ----
            whh = pc.tile([128, 4, 4 * H], dt_w)
            nc.sync.dma_start(whh[:], whh_d)
            wih = pc.tile([33, 4 * H], dt_w)
            nc.sync.dma_start(wih[:], wih_d)
            w1s = pc.tile([128, 4, M1], dt_w)
            nc.sync.dma_start(w1s[:], w1_d)
            w2s = pc.tile([128, 4, M2], dt_w)
            nc.sync.dma_start(w2s[:], w2_d)
            wms = pc.tile([128, 2, 64], dt_w)
            nc.sync.dma_start(wms[:], wms_d)
            lng = pc.tile([128, 4], F32)
            nc.sync.dma_start(lng[:], lng_d)
            lnb = pc.tile([128, 4], F32)
            nc.sync.dma_start(lnb[:], lnb_d)
            b1s = pc.tile([128, 4], F32)
            nc.sync.dma_start(b1s[:], b1_d)
            nb1s = pc.tile([128, 4], F32)
            nc.sync.dma_start(nb1s[:], nb1_d)
            b2s = pc.tile([128, 2], F32)
            nc.sync.dma_start(b2s[:], b2_d)
            nb2s = pc.tile([128, 2], F32)
            nc.sync.dma_start(nb2s[:], nb2_d)
            csta = pc.tile([64, 2], F32)
            nc.sync.dma_start(csta[:], csta_d)
            cstb = pc.tile([1, 64], dt_w)
            nc.sync.dma_start(cstb[:], cstb_d)
            cstc = pc.tile([2, 1], F32)
            nc.sync.dma_start(cstc[:], cstc_d)
            h0 = pc.tile([128, 4, BL], F32)
            nc.sync.dma_start(h0[:], h0_d)
            c0 = pc.tile([128, 4, BL], F32)
            nc.sync.dma_start(c0[:], c0_d)

            ones_mu = pc.tile([128, 1], F32)
            nc.vector.memset(ones_mu[:], 1.0 / H)
            ones_bc = pc.tile([1, 128], F32)
            nc.vector.memset(ones_bc[:], 1.0)
            ones_row = pc.tile([1, WC], dt_w)
            nc.vector.memset(ones_row[:], 1.0)

            # ---- gx window fill: gx.T = W_ihb @ [x;1].T  -> SBUF ----
            def gx_fill(w):
                gxw = pgxw.tile([128, W, 512], dt_w, tag="gxw")
                xa = pio.tile([33, WC], dt_w, tag="xa")
                nc.sync.dma_start(xa[:], x_d[:, w * WC:(w + 1) * WC])
                for m in range(16):
                    ps = psgx.tile([128, WC], F32, tag="gx")
                    nc.tensor.matmul(ps[:], wih[:, bass.ts(m, 128)], xa[:],
                                     start=True, stop=True)
                    nc.scalar.copy(
                        gxw[:, :, bass.ts(m, 32)],
                        ps[:].rearrange("p (s b) -> p s b", s=W))
                return gxw

            # ---- one LSTM step ----
            def lstm_step(t, h_prev, c_prev, gxw, hsw):
                toff = t % W
                mk = pmk.tile([128, 128], F32, tag="mk")
                nc.sync.dma_start(mk[:], mk_d[t])
                mk3 = mk[:].rearrange("p (q b) -> p q b", q=4)
                ht = pstep.tile([128, 4, BL], dt_w, tag="ht")
                nc.vector.tensor_tensor(ht[:], h_prev, mk3, OP.mult)
                ct = pstep.tile([128, 4, BL], F32, tag="ct")
                nc.vector.tensor_tensor(ct[:], c_prev, mk3, OP.mult)

                G = psg.tile([128, 4, 128], F32, tag="G")
                for m in range(16):
                    q, r = divmod(m, 4)
                    for k in range(4):
                        nc.tensor.matmul(
                            G[:, q, bass.ts(r, 32)],
                            whh[:, k, bass.ts(m, 128)],
                            ht[:, k, :],
                            start=(k == 0), stop=(k == 3))

                act = pstep.tile([128, 4, 128], F32, tag="act")
                gx3 = gxw[:, toff, :].rearrange("p (q j) -> p q j", q=4)
                nc.vector.tensor_tensor(act[:], G[:], gx3, OP.add)
                nc.scalar.activation(act[:, :, 0:96], act[:, :, 0:96], AF.Sigmoid)
                nc.scalar.activation(act[:, :, 96:128], act[:, :, 96:128], AF.Tanh)
                i_s = act[:, :, 0:32]
                f_s = act[:, :, 32:64]
                o_s = act[:, :, 64:96]
                tg = act[:, :, 96:128]

                t1 = pstep.tile([128, 4, BL], F32, tag="t1")
                nc.vector.tensor_tensor(t1[:], f_s, ct[:], OP.mult)
                t2 = pstep.tile([128, 4, BL], F32, tag="t2")
                nc.vector.tensor_tensor(t2[:], i_s, tg, OP.mult)
                c_new = pst.tile([128, 4, BL], F32, tag="c")
                nc.vector.tensor_tensor(c_new[:], t1[:], t2[:], OP.add)
                tcx = pstep.tile([128, 4, BL], F32, tag="tc")
                nc.scalar.activation(tcx[:], c_new[:], AF.Tanh)
                h_slot = hsw[:, :, toff, :]
                nc.vector.tensor_tensor(h_slot, o_s, tcx[:], OP.mult)
                return h_slot, c_new

            # ---- per-window LayerNorm + MLP + heads ----
            def win_phase(w, hsw):
                # LN stats via ones-matmul partition reduce
                sq = pbig.tile([128, 4, W, BL], F32, tag="sq")
                for k in range(4):
                    nc.scalar.activation(sq[:, k], hsw[:, k], AF.Square)
                mu = psw.tile([1, WC], F32, tag="w")
                for k in range(4):
                    nc.tensor.matmul(mu[:], ones_mu[:],
                                     hsw[:, k].rearrange("p s b -> p (s b)"),
                                     start=(k == 0), stop=(k == 3))
                ms = psw.tile([1, WC], F32, tag="w")
                for k in range(4):
                    nc.tensor.matmul(ms[:], ones_mu[:],
                                     sq[:, k].rearrange("p s b -> p (s b)"),
                                     start=(k == 0), stop=(k == 3))
                murow = psm.tile([1, WC], F32, tag="murow")
                nc.scalar.copy(murow[:], mu[:])
                mu2 = psm.tile([1, WC], F32, tag="mu2")
                nc.scalar.square(mu2[:], mu[:])
                ve = psm.tile([1, WC], F32, tag="ve")
                nc.vector.tensor_tensor(ve[:], ms[:], mu2[:], OP.subtract)
                nc.vector.tensor_scalar(ve[:], ve[:], EPS, None, OP.add)
                rec = psm.tile([1, WC], F32, tag="rec")
                nc.vector.reciprocal(rec[:], ve[:])
                rstd = psm.tile([1, WC], F32, tag="rstd")
                nc.scalar.sqrt(rstd[:], rec[:])

                bcm = psw.tile([128, WC], F32, tag="w")
                nc.tensor.matmul(bcm[:], ones_bc[:], murow[:], start=True, stop=True)
                bcr = psw.tile([128, WC], F32, tag="w")
                nc.tensor.matmul(bcr[:], ones_bc[:], rstd[:], start=True, stop=True)

                hn = pbig.tile([128, 4, WC], dt_w, tag="hn")
                for k in range(4):
                    tmp = psm.tile([128, WC], F32, tag="lntmp")
                    nc.vector.tensor_tensor(
                        tmp[:], hsw[:, k].rearrange("p s b -> p (s b)"),
                        bcm[:], OP.subtract)
                    nc.vector.tensor_tensor(tmp[:], tmp[:], bcr[:], OP.mult)
                    nc.scalar.activation(hn[:, k], tmp[:], AF.Identity,
                                         bias=lnb[:, k:k + 1], scale=lng[:, k:k + 1])

                # MLP layer 1 (+ELU; output is elu(y)+1, -1 folded into b2')
                h1 = pbig.tile([128, 4, WC], dt_w, tag="h1")
                for m in range(4):
                    ps = psw.tile([128, WC], F32, tag="w")
                    for k in range(4):
                        nc.tensor.matmul(ps[:], w1s[:, k, bass.ts(m, 128)],
                                         hn[:, k], start=(k == 0), stop=(k == 3))
                    r = psm.tile([128, WC], F32, tag="er")
                    nc.scalar.activation(r[:], ps[:], AF.Relu, bias=b1s[:, m:m + 1])
                    r2 = psm.tile([128, WC], F32, tag="er2")
                    nc.scalar.activation(r2[:], ps[:], AF.Relu,
                                         bias=nb1s[:, m:m + 1], scale=-1.0)
                    e = psm.tile([128, WC], F32, tag="ee")
                    nc.scalar.activation(e[:], r2[:], AF.Exp, scale=-1.0)
                    nc.vector.tensor_tensor(h1[:, m], e[:], r[:], OP.add)

                # MLP layer 2
                h2 = pbig.tile([128, 2, WC], dt_w, tag="h2")
                for m in range(2):
                    ps = psw.tile([128, WC], F32, tag="w")
                    for k in range(4):
                        nc.tensor.matmul(ps[:], w2s[:, k, bass.ts(m, 128)],
                                         h1[:, k], start=(k == 0), stop=(k == 3))
                    r = psm.tile([128, WC], F32, tag="er")
                    nc.scalar.activation(r[:], ps[:], AF.Relu, bias=b2s[:, m:m + 1])
                    r2 = psm.tile([128, WC], F32, tag="er2")
                    nc.scalar.activation(r2[:], ps[:], AF.Relu,
                                         bias=nb2s[:, m:m + 1], scale=-1.0)
                    e = psm.tile([128, WC], F32, tag="ee")
                    nc.scalar.activation(e[:], r2[:], AF.Exp, scale=-1.0)
                    nc.vector.tensor_tensor(h2[:, m], e[:], r[:], OP.add)

                # heads: [mean; logstd_raw] = Wms @ h2' + b (bias via ones-row MM)
                hd = psw.tile([64, WC], F32, tag="w")
                for k in range(2):
                    nc.tensor.matmul(hd[:], wms[:, k], h2[:, k],
                                     start=(k == 0), stop=False)
                nc.tensor.matmul(hd[:], cstb[:], ones_row[:],
                                 start=False, stop=True)

                meanv = pio.tile([A, WC], F32, tag="mean")
                nc.scalar.copy(meanv[:], hd[0:A, :])
                stack = psm.tile([64, WC], F32, tag="stack")
                nc.vector.memset(stack[:], 0.0)
                nc.vector.tensor_scalar(stack[32:32 + A, :], hd[32:32 + A, :],
                                        -5.0, 2.0, OP.max, OP.min)
                stdv = pio.tile([A, WC], F32, tag="std")
                nc.scalar.activation(stdv[:], stack[32:32 + A, :], AF.Exp)
                rs = psm.tile([A, WC], F32, tag="rs")
                nc.scalar.activation(rs[:], stack[32:32 + A, :], AF.Exp, scale=-1.0)
                at = pio.tile([A, WC], F32, tag="at")
                nc.sync.dma_start(at[:], a_d[:, w * WC:(w + 1) * WC])
                z = psm.tile([A, WC], F32, tag="z")
                nc.vector.tensor_tensor(z[:], at[:], meanv[:], OP.subtract)
                nc.vector.tensor_tensor(z[:], z[:], rs[:], OP.mult)
                nc.scalar.activation(stack[0:A, :], z[:], AF.Square)

                le = psw.tile([2, WC], F32, tag="w")
                nc.tensor.matmul(le[:], csta[:], stack[:], start=True, stop=True)
                leo = pio.tile([2, WC], F32, tag="leo")
                nc.vector.tensor_scalar(leo[:], le[:], cstc[:, 0:1], None, OP.add)
                nc.sync.dma_start(le_o[:, w * WC:(w + 1) * WC], leo[:])
                nc.sync.dma_start(mean_o[:, w * WC:(w + 1) * WC], meanv[:])
                nc.sync.dma_start(std_o[:, w * WC:(w + 1) * WC], stdv[:])

            # ---- main emission ----
            gx_tiles = {0: gx_fill(0)}
            if nw > 1:
                gx_tiles[1] = gx_fill(1)
            h_prev, c_prev = h0[:], c0[:]
            for w in range(nw):
                hsw = phsw.tile([128, 4, W, BL], F32, tag="hsw")
                for toff in range(W):
                    t = w * W + toff
                    h_prev, c_new = lstm_step(t, h_prev, c_prev, gx_tiles[w], hsw)
                    c_prev = c_new[:]
                if w + 2 < nw:
                    gx_tiles[w + 2] = gx_fill(w + 2)
                    del gx_tiles[w]
                win_phase(w, hsw)
                if w == nw - 1:
                    nc.sync.dma_start(hT_o, h_prev)
                    nc.sync.dma_start(cT_o, c_prev)

    nc.compile()
    return nc


_BUILD_CACHE = {}


def _get_nc(nt, dt_w=DT_W):
    key = (nt, str(dt_w))
    if key not in _BUILD_CACHE:
        _BUILD_CACHE[key] = build(nt, dt_w)
    return _BUILD_CACHE[key]


def _prep_core_inputs(c, nt, x, done, h0, c0, action,
                      W_ih, W_hh, b_ih, b_hh, g_ln, b_ln,
                      W1, b1, W2, b2, Wm, bm, Ws, bs, dt_w=DT_W):
    perm = _gate_perm()
    import ml_dtypes
    npdt = np.float32 if dt_w == F32 else ml_dtypes.bfloat16
    bsl = slice(BL * c, BL * (c + 1))

    W_hh_re = W_hh[perm].astype(np.float32)
    W_ih_re = W_ih[perm].astype(np.float32)
    bias_re = (b_ih + b_hh)[perm].astype(np.float32)

    whhT = np.ascontiguousarray(
        W_hh_re.T.reshape(4, 128, 4 * H).transpose(1, 0, 2)).astype(npdt)
    wihbT = np.ascontiguousarray(
        np.concatenate([W_ih_re, bias_re[:, None]], 1).T).astype(npdt)

    xc = x[:nt, bsl, :]                                   # [nt, BL, OBS]
    xaugT = np.empty((33, nt * BL), dtype=npdt)
    xaugT[:32] = xc.transpose(2, 0, 1).reshape(OBS, nt * BL)
    xaugT[32] = 1.0

    mrow = (1.0 - done[:nt, bsl]).astype(np.float32)      # [nt, BL]
    maskb = np.broadcast_to(
        np.tile(mrow, (1, 4))[:, None, :], (nt, 128, 128))
    maskb = np.ascontiguousarray(maskb)

    aT = np.ascontiguousarray(
        action.reshape(T, B, A)[:nt, bsl].transpose(2, 0, 1).reshape(A, nt * BL))

    h0T = np.ascontiguousarray(
        h0[0, bsl].T.reshape(4, 128, BL).transpose(1, 0, 2))
    c0T = np.ascontiguousarray(
        c0[0, bsl].T.reshape(4, 128, BL).transpose(1, 0, 2))

    w1T = np.ascontiguousarray(
        W1.T.reshape(4, 128, M1).transpose(1, 0, 2)).astype(npdt)
    w2T = np.ascontiguousarray(
        W2.T.reshape(4, 128, M2).transpose(1, 0, 2)).astype(npdt)
    wms_pad = np.zeros((64, M2), dtype=np.float32)
    wms_pad[0:A] = Wm
    wms_pad[32:32 + A] = Ws
    wmsT = np.ascontiguousarray(
        wms_pad.T.reshape(2, 128, 64).transpose(1, 0, 2)).astype(npdt)

    lng = np.ascontiguousarray(g_ln.reshape(4, 128).T)
    lnb = np.ascontiguousarray(b_ln.reshape(4, 128).T)
    b1v = np.ascontiguousarray(b1.reshape(4, 128).T)
    b2p = b2 - W2.sum(1)                                   # fold elu +1 of h1
    b2v = np.ascontiguousarray(b2p.reshape(2, 128).T)
    bmp = bm - Wm.sum(1)                                   # fold elu +1 of h2
    bsp = bs - Ws.sum(1)

    cst_a = np.zeros((64, 2), dtype=np.float32)
    cst_a[0:A, 0] = -0.5
    cst_a[32:32 + A, 0] = -1.0
    cst_a[32:32 + A, 1] = 1.0
    cst_b = np.zeros((1, 64), dtype=np.float32)
    cst_b[0, 0:A] = bmp
    cst_b[0, 32:32 + A] = bsp
    cst_b = cst_b.astype(npdt)
    cst_c = np.array([[-0.5 * A * LOG2PI], [A * (0.5 + 0.5 * LOG2PI)]],
                     dtype=np.float32)

    return {
        "xaugT": xaugT, "maskb": maskb, "aT": aT,
        "h0T": h0T, "c0T": c0T,
        "whhT": whhT, "wihbT": wihbT,
        "w1T": w1T, "w2T": w2T, "wmsT": wmsT,
        "lng": lng, "lnb": lnb,
        "b1": b1v, "nb1": -b1v, "b2": b2v, "nb2": -b2v,
        "cst_a": cst_a, "cst_b": cst_b, "cst_c": cst_c,
    }


def run_device(nt, inputs, dt_w=DT_W, trace=False):
    """Shard, run on 8 cores, return per-core result dicts."""
    nc = _get_nc(nt, dt_w)
    in_maps = [
        _prep_core_inputs(c, nt, dt_w=dt_w, **inputs) for c in range(NCORES)
    ]
    res = run_bass_kernel_spmd(nc, in_maps, core_ids=list(range(NCORES)),
                               trace=trace)
    return res


def assemble(nt, inputs, results):
    """Gather per-core outputs into full reference-shaped outputs."""
    ntb = nt * B
    logp = np.empty((nt, B), dtype=np.float32)
    ent = np.empty((nt, B), dtype=np.float32)
    mean = np.empty((nt, B, A), dtype=np.float32)
    std = np.empty((nt, B, A), dtype=np.float32)
    h1 = np.empty((1, B, H), dtype=np.float32)
    c1 = np.empty((1, B, H), dtype=np.float32)
    for c in range(NCORES):
        bsl = slice(BL * c, BL * (c + 1))
        r = results[c]
        logp[:, bsl] = r["le"][0].reshape(nt, BL)
        ent[:, bsl] = r["le"][1].reshape(nt, BL)
        mean[:, bsl] = r["meanT"].reshape(A, nt, BL).transpose(1, 2, 0)
        std[:, bsl] = r["stdT"].reshape(A, nt, BL).transpose(1, 2, 0)
        h1[0, bsl] = r["hT"].transpose(2, 1, 0).reshape(BL, H)
        c1[0, bsl] = r["cT"].transpose(2, 1, 0).reshape(BL, H)
    action = np.asarray(inputs["action"], dtype=np.float32)[:ntb]
    return (action, logp.reshape(ntb), ent.reshape(ntb), (h1, c1),
            mean.reshape(ntb, A), std.reshape(ntb, A))


def kernel(**inputs):
    """Full unsharded inputs in, full reference-shaped output out."""
    inputs = {k: np.asarray(v) for k, v in inputs.items()}
    res = run_device(T, inputs)
    return assemble(T, inputs, res.results)


if __name__ == "__main__":
    rng = np.random.default_rng(0)
    ins = {
        "x": rng.standard_normal((T, B, OBS), dtype=np.float32),
        "done": (rng.random((T, B)) < 0.05).astype(np.float32),
        "h0": rng.standard_normal((1, B, H), dtype=np.float32) * 0.1,
        "c0": rng.standard_normal((1, B, H), dtype=np.float32) * 0.1,
        "action": rng.standard_normal((T * B, A), dtype=np.float32),
        "W_ih": rng.standard_normal((4 * H, OBS), dtype=np.float32) * 0.02,
        "W_hh": rng.standard_normal((4 * H, H), dtype=np.float32) * 0.02,
        "b_ih": np.zeros(4 * H, np.float32),
        "b_hh": np.zeros(4 * H, np.float32),
        "g_ln": np.ones(H, np.float32),
        "b_ln": np.zeros(H, np.float32),
        "W1": rng.standard_normal((M1, H), dtype=np.float32) * 0.02,
        "b1": np.zeros(M1, np.float32),
        "W2": rng.standard_normal((M2, M1), dtype=np.float32) * 0.02,
        "b2": np.zeros(M2, np.float32),
        "Wm": rng.standard_normal((A, M2), dtype=np.float32) * 0.01,
        "bm": np.zeros(A, np.float32),
        "Ws": rng.standard_normal((A, M2), dtype=np.float32) * 0.01,
        "bs": np.zeros(A, np.float32),
    }
    out = kernel(**ins)
    print("kernel ran; logp[:4] =", out[1][:4])
